# revision 59
# baseline (speedup 1.0000x reference)
"""GPTQ int4 quant linear: y = x @ dequant(qweight) + bias on 8 TRN2 cores.

Sharding: 2-way over tokens x 4-way over out_features (core c = (ti, oj)).
Host pre-transposes x to xT [in_f, tok] in bf16 with k rows permuted into
nibble-unpack order (pure layout prep), so the device kernel needs NO PE
transposes: every tensor-engine instruction is a real N=512 matmul (2048
per core, the floor given the 512-f32 PSUM bank limit; measured ~505us
per core, PE busy ~449us vs a 437us bf16 streaming floor). Scales ship
as bf16 so broadcast expands read the input DRAM tensor directly.

Dequant (all DVE, ~2.2us/k-tile): one i32 shift+mask extracts nibble
planes j and j+4 into the low/high u16 lanes at once, then per plane a
tensor_tensor subtract reads the strided u16 lanes directly (int->bf16
conversion inside the op) against the broadcast (z+1) tile, and a packed
bf16 multiply applies the scale. 32 dequantized k-tiles stay resident in
SBUF. The t=0 group/partition broadcasts ride two tiny PE matmuls with a
0/1 selector (DMA completion semaphores lag ~5-10us and would gate
W[0]); later groups use DMA expands off the critical path. Input DMAs
are spread across the sync/scalar/gpsimd queues for the same reason.

Main loop: token blocks sized [4]*7 + [2,2] (4 x 2 PSUM banks in
flight; the small tail blocks shrink the final drain latency). Per
block, k-major: stream xT chunks [128k, blk*128tok] (prefetch distance
4) and accumulate into the PSUM tiles; drain = bias add on DVE, out-DMA
on alternating scalar/gpsimd queues. Host assembles the 2x4 output grid.
"""

import numpy as np
import ml_dtypes

import concourse.bass as bass
import concourse.mybir as mybir
import concourse.tile as tile
from concourse import bacc

F32 = mybir.dt.float32
I32 = mybir.dt.int32
U16 = mybir.dt.uint16
BF16 = mybir.dt.bfloat16

N_CORES = 8
N_TOK_SHARDS = 2
N_OUT_SHARDS = 4
TOK = 8192
IN_F = 4096
OUT_F = 4096
TOK_SH = TOK // N_TOK_SHARDS  # 4096
OUT_SH = OUT_F // N_OUT_SHARDS  # 1024
PACKED_K = IN_F // 8  # 512 packed rows
GROUPSIZE = 128
N_GROUPS = IN_F // GROUPSIZE  # 32
P = 128
BLOCK_SIZES = [4] * 7 + [2, 2]  # token tiles per block
PF = 4  # x-chunk prefetch distance

ALU = mybir.AluOpType


def build_nc(tok=TOK_SH):
    n_mtiles = tok // P  # 32
    assert sum(BLOCK_SIZES) == n_mtiles
    n_t = PACKED_K // P  # 4 packed-row tiles
    n_kt = n_t * 8  # 32 k-tiles
    blk_off = np.cumsum([0] + BLOCK_SIZES)
    nc = bacc.Bacc(None, target_bir_lowering=False)

    # x arrives host-transposed (k in nibble-unpack order) and bf16
    xT = nc.dram_tensor("x", [IN_F, tok], BF16, kind="ExternalInput")
    qw = nc.dram_tensor("qw", [PACKED_K, OUT_SH], I32, kind="ExternalInput")
    qz = nc.dram_tensor("qz", [N_GROUPS, OUT_SH // 8], I32, kind="ExternalInput")
    # scales arrive column-doubled from the host: sc[g, 2n+p] = scale[g, n]
    sc = nc.dram_tensor("sc", [N_GROUPS, 2 * OUT_SH], BF16, kind="ExternalInput")
    bi = nc.dram_tensor("bi", [1, OUT_SH], F32, kind="ExternalInput")
    out = nc.dram_tensor("out", [tok, OUT_SH], F32, kind="ExternalOutput")

    with tile.TileContext(nc) as tc:
        with (
            tc.tile_pool(name="singles", bufs=1) as singles,
            tc.tile_pool(name="weights", bufs=1) as wpool,
            tc.tile_pool(name="qwin", bufs=1) as qwpool,
            tc.tile_pool(name="nib", bufs=3) as nibpool,
            tc.tile_pool(name="scexp", bufs=2) as scpool,
            tc.tile_pool(name="xin", bufs=PF + 2) as xpool,
            tc.tile_pool(name="yout", bufs=4) as ypool,
            tc.tile_pool(name="psum_y", bufs=4, space="PSUM") as psum_y,
        ):
            # ---- input DMAs, spread across queues (single-queue DMA
            # bandwidth + completion-semaphore lag would serialize the
            # W[0] critical path otherwise); qz first ----
            qz_sb = singles.tile([N_GROUPS, OUT_SH // 8], I32)
            nc.gpsimd.dma_start(qz_sb, qz[:, :])

            # 0/1 selector for the group->partition broadcast done on the
            # PE: exp_t[p, n] = sum_g E[g, 128t+p] * src[g, n] replicates
            # group row 8t + p//16 across the 16 partitions that use it.
            e_np = np.zeros((N_GROUPS, n_t * P), dtype=np.float32)
            for t in range(n_t):
                for p in range(P):
                    e_np[8 * t + p // 16, t * P + p] = 1.0
            e_dram = nc.inline_tensor(
                e_np.astype(ml_dtypes.bfloat16), name="gsel"
            )
            e_sb = singles.tile([N_GROUPS, n_t * P], BF16)
            nc.gpsimd.dma_start(e_sb, e_dram[:, :])

            # scale expands own the scalar queue head: sc0's completion
            # semaphore gates W[0], and DMA sems lag ~5-10us behind queue
            # position, so nothing else may precede them on this queue
            scale_exps = []
            for t in range(n_t):
                scale_exp2 = scpool.tile([P, 2 * OUT_SH], BF16, tag="scale_exp")
                nc.scalar.dma_start(
                    out=scale_exp2,
                    in_=bass.AP(
                        tensor=sc,
                        offset=t * 8 * 2 * OUT_SH,
                        ap=[[2 * OUT_SH, 8], [0, 16], [1, 2 * OUT_SH]],
                    ),
                )
                scale_exps.append(scale_exp2)

            qw_tiles = []
            for t in range(n_t):
                qw_t = qwpool.tile([P, OUT_SH], I32, tag=f"qw{t}")
                nc.gpsimd.dma_start(qw_t, qw[t * P : (t + 1) * P, :])
                qw_tiles.append(qw_t)

            bias_sb = singles.tile([P, OUT_SH], F32)
            nc.gpsimd.dma_start(out=bias_sb, in_=bi[:, :].to_broadcast((P, OUT_SH)))

            # x chunk loads: chunk (b, kt) = xT[128 k, blk*128 tok]
            x_tiles = {}

            def load_chunk(b, kt):
                bs = BLOCK_SIZES[b]
                t0 = int(blk_off[b]) * P
                x_t = xpool.tile([P, bs * P], BF16, tag="x")
                nc.sync.dma_start(
                    x_t, xT[kt * P : (kt + 1) * P, t0 : t0 + bs * P]
                )
                x_tiles[(b, kt)] = x_t

            order = [(b, kt) for b in range(len(BLOCK_SIZES)) for kt in range(n_kt)]
            for i in range(PF):
                load_chunk(*order[i])

            # ---- zero-point prep: zq1[g, n] = (qz nibbles) + 1, bf16 ----
            zq1_i = singles.tile([N_GROUPS, OUT_SH], I32)
            zq1_i_r = zq1_i.rearrange("g (m j) -> g m j", j=8)
            for j in range(8):
                nc.vector.tensor_scalar(
                    out=zq1_i_r[:, :, j],
                    in0=qz_sb[:, :],
                    scalar1=4 * j,
                    scalar2=0xF,
                    op0=ALU.logical_shift_right,
                    op1=ALU.bitwise_and,
                )
            zq1_bf = singles.tile([N_GROUPS, OUT_SH], BF16)
            nc.vector.tensor_scalar(
                out=zq1_bf, in0=zq1_i, scalar1=1, scalar2=None, op0=ALU.add
            )
            # column-doubled copy: source for the t1..3 DMA expands
            zq1_bf2 = singles.tile([N_GROUPS, 2 * OUT_SH], BF16)
            nc.vector.tensor_scalar(
                out=zq1_bf2.rearrange("g (n two) -> g n two", two=2),
                in0=bass.AP(
                    tensor=zq1_bf.tensor,
                    offset=zq1_bf.offset,
                    ap=[[OUT_SH, N_GROUPS], [1, OUT_SH], [0, 2]],
                ),
                scalar1=0,
                scalar2=None,
                op0=ALU.add,
            )

            # ---- dequantize weight shard into 32 resident bf16 tiles ----
            # tile (t, j) holds W rows k = 8*kk + j, kk in [128t, 128t+128);
            # partition kk maps to group 8t + (kk % 128)//16, hence the
            # 8-groups x 16-reps broadcast expansion.
            # Dequant: the DVE keeps nibble planes j and j+4 interleaved in
            # u16 lanes (all ops 2-byte packed -> 2x rate, ~1.56us/k-tile);
            # the otherwise-idle Scalar engine de-interleaves each plane
            # into a packed resident W tile (~1.2us/k-tile). Both rates
            # beat the PE's 1.7us/k-tile consumption, so the PE never
            # starves waiting for weights, and the matmul rhs stays packed.
            w_views = [None] * n_kt
            for t in range(n_t):
                zq1_exp2 = scpool.tile([P, 2 * OUT_SH], BF16, tag="zq1_exp")
                scale_exp2 = scale_exps[t]
                if t == 0:
                    # t0's zq1 is on the W[0] critical path: broadcast via
                    # two tiny PE matmuls + a column-doubling scalar copy
                    # instead of a DMA expand (completion-semaphore lag).
                    ep = psum_y.tile([P, OUT_SH], F32, tag="y")
                    for h in range(2):
                        nc.tensor.matmul(
                            ep[:, h * 512 : (h + 1) * 512],
                            lhsT=e_sb[:, t * P : (t + 1) * P],
                            rhs=zq1_bf[:, h * 512 : (h + 1) * 512],
                        )
                    nc.scalar.copy(
                        zq1_exp2.rearrange("p (n two) -> p n two", two=2),
                        bass.AP(
                            tensor=ep.tensor,
                            offset=ep.offset,
                            ap=[[OUT_SH, P], [1, OUT_SH], [0, 2]],
                        ),
                    )
                else:
                    # off the critical path: SBUF->SBUF DMA expand from the
                    # doubled zq1 copy
                    nc.gpsimd.dma_start(
                        out=zq1_exp2,
                        in_=bass.AP(
                            tensor=zq1_bf2.tensor,
                            offset=zq1_bf2.offset + t * 8 * 2 * OUT_SH,
                            ap=[[2 * OUT_SH, 8], [0, 16], [1, 2 * OUT_SH]],
                        ),
                    )
                qw_t = qw_tiles[t]
                for jj in range(4):
                    # Dual-plane extract: nibbles jj and jj+4 sit 16 bits
                    # apart, so one i32 shift+mask yields both planes in the
                    # low/high u16 lanes.
                    nib2 = nibpool.tile([P, OUT_SH], I32, tag="nib")
                    nc.vector.tensor_scalar(
                        out=nib2,
                        in0=qw_t,
                        scalar1=4 * jj,
                        scalar2=0x000F000F,
                        op0=ALU.logical_shift_right,
                        op1=ALU.bitwise_and,
                    )
                    if t == 0 and jj == 0:
                        # fast path for the first pair: strided per-plane
                        # ops write W[0]/W[4] directly (shortest chain to
                        # the first matmul -- no extra engine hops)
                        nib2u = nib2[:, :].bitcast(U16).rearrange(
                            "p (n two) -> p n two", two=2
                        )
                        zq1v = zq1_exp2.rearrange(
                            "p (n two) -> p n two", two=2
                        )
                        scv = scale_exp2.rearrange(
                            "p (n two) -> p n two", two=2
                        )
                        for half in range(2):
                            kt = jj + 4 * half
                            d = nibpool.tile([P, OUT_SH], BF16, tag="dfp")
                            nc.vector.tensor_tensor(
                                out=d,
                                in0=nib2u[:, :, half],
                                in1=zq1v[:, :, 0],
                                op=ALU.subtract,
                            )
                            w = wpool.tile([P, OUT_SH], BF16, tag=f"w{kt}")
                            nc.vector.tensor_tensor(
                                out=w, in0=d, in1=scv[:, :, 0], op=ALU.mult
                            )
                            w_views[kt] = [
                                w[:, h * 512 : (h + 1) * 512]
                                for h in range(2)
                            ]
                        continue
                    d2 = nibpool.tile([P, 2 * OUT_SH], BF16, tag="d")
                    nc.vector.tensor_tensor(
                        out=d2,
                        in0=nib2[:, :].bitcast(U16),
                        in1=zq1_exp2,
                        op=ALU.subtract,
                    )
                    w2 = nibpool.tile([P, 2 * OUT_SH], BF16, tag="wp")
                    nc.vector.tensor_tensor(
                        out=w2, in0=d2, in1=scale_exp2, op=ALU.mult
                    )
                    w2v = w2.rearrange("p (n two) -> p n two", two=2)
                    for half in range(2):
                        kt = t * 8 + jj + 4 * half
                        w = wpool.tile([P, OUT_SH], BF16, tag=f"w{kt}")
                        nc.scalar.copy(w, w2v[:, :, half])
                        w_views[kt] = [
                            w[:, h * 512 : (h + 1) * 512] for h in range(2)
                        ]

            # ---- main loop: token blocks, k-major inside ----
            ndma = 0
            for b, bs in enumerate(BLOCK_SIZES):
                yps = []
                for i in range(bs):
                    yp = psum_y.tile([P, OUT_SH], F32, tag="y")
                    yps.append(yp)
                for kt in range(n_kt):
                    pos = b * n_kt + kt
                    if pos + PF < len(order):
                        load_chunk(*order[pos + PF])
                    xt = x_tiles.pop((b, kt))
                    for i in range(bs):
                        for h in range(2):
                            nc.tensor.matmul(
                                yps[i][:, h * 512 : (h + 1) * 512],
                                lhsT=xt[:, i * P : (i + 1) * P],
                                rhs=w_views[kt][h],
                                start=(kt == 0),
                                stop=(kt == n_kt - 1),
                            )
                for i in range(bs):
                    mi = int(blk_off[b]) + i
                    y_sb = ypool.tile([P, OUT_SH], F32, tag="y_sb")
                    nc.vector.tensor_add(y_sb, yps[i], bias_sb)
                    eng = nc.scalar if ndma % 2 == 0 else nc.gpsimd
                    eng.dma_start(out[mi * P : (mi + 1) * P, :], y_sb)
                    ndma += 1

    nc.compile()
    return nc


_NC_CACHE = {}


def _get_nc(tok=TOK_SH):
    if tok not in _NC_CACHE:
        _NC_CACHE[tok] = build_nc(tok)
    return _NC_CACHE[tok]


def _shard_inputs(x, qweight, qzeros, scales, bias, tok_sh=TOK_SH):
    # Device W tile (t, j) row r holds original k = 1024t + 8r + j (nibble
    # unpack order), i.e. device row d = 1024t + 128j + r. Permute x's k
    # axis to match while transposing to [k, tok] bf16.
    ntok = x.shape[0]
    xT = np.ascontiguousarray(
        np.asarray(x, dtype=np.float32)
        .reshape(ntok, 4, 128, 8)
        .transpose(1, 3, 2, 0)
        .reshape(IN_F, ntok)
        .astype(ml_dtypes.bfloat16)
    )
    sc_bf = np.repeat(
        np.asarray(scales, dtype=np.float32).astype(ml_dtypes.bfloat16), 2, axis=1
    )
    in_maps = []
    for c in range(N_CORES):
        ti, oj = divmod(c, N_OUT_SHARDS)
        sl = slice(oj * OUT_SH, (oj + 1) * OUT_SH)
        slz = slice(oj * (OUT_SH // 8), (oj + 1) * (OUT_SH // 8))
        in_maps.append(
            {
                "x": np.ascontiguousarray(
                    xT[:, ti * tok_sh : (ti + 1) * tok_sh]
                ),
                "qw": np.ascontiguousarray(qweight[:, sl], dtype=np.int32),
                "qz": np.ascontiguousarray(qzeros[:, slz], dtype=np.int32),
                "sc": np.ascontiguousarray(
                    sc_bf[:, oj * 2 * OUT_SH : (oj + 1) * 2 * OUT_SH]
                ),
                "bi": np.ascontiguousarray(
                    bias[sl].reshape(1, OUT_SH), dtype=np.float32
                ),
            }
        )
    return in_maps


def _assemble(per_core, tok_sh=TOK_SH):
    out = np.empty((N_TOK_SHARDS * tok_sh, OUT_F), dtype=np.float32)
    for c in range(N_CORES):
        ti, oj = divmod(c, N_OUT_SHARDS)
        out[ti * tok_sh : (ti + 1) * tok_sh, oj * OUT_SH : (oj + 1) * OUT_SH] = (
            per_core[c]["out"]
        )
    return out


class PjrtRunner:
    """Builds the shard_map'd bass executable once; supports timed re-runs."""

    def __init__(self, nc):
        import jax
        from jax.sharding import Mesh, PartitionSpec
        from jax.experimental.shard_map import shard_map
        from concourse import bass2jax, mybir as mb

        self.jax = jax
        bass2jax.install_neuronx_cc_hook()

        partition_name = (
            nc.partition_id_tensor.name if nc.partition_id_tensor else None
        )
        in_names, out_names, out_avals, zero_outs = [], [], [], []
        for alloc in nc.m.functions[0].allocations:
            if not isinstance(alloc, mb.MemoryLocationSet):
                continue
            name = alloc.memorylocations[0].name
            if alloc.kind == "ExternalInput":
                if name != partition_name:
                    in_names.append(name)
            elif alloc.kind == "ExternalOutput":
                shape = tuple(alloc.tensor_shape)
                dtype = mb.dt.np(alloc.dtype)
                out_names.append(name)
                out_avals.append(jax.core.ShapedArray(shape, dtype))
                zero_outs.append(np.zeros(shape, dtype))
        self.in_names = in_names
        self.out_names = out_names
        self.zero_outs = zero_outs
        n_params = len(in_names)
        all_in_names = in_names + out_names
        if partition_name is not None:
            all_in_names.append(partition_name)

        def _body(*args):
            operands = list(args)
            if partition_name is not None:
                operands.append(bass2jax.partition_id_tensor())
            outs = bass2jax._bass_exec_p.bind(
                *operands,
                out_avals=tuple(out_avals),
                in_names=tuple(all_in_names),
                out_names=tuple(out_names),
                lowering_input_output_aliases=(),
                sim_require_finite=True,
                sim_require_nnan=True,
                nc=nc,
            )
            return tuple(outs)

        devices = jax.devices()[:N_CORES]
        self.mesh = Mesh(np.asarray(devices), ("core",))
        in_specs = (PartitionSpec("core"),) * (n_params + len(out_names))
        out_specs = (PartitionSpec("core"),) * len(out_names)
        # no donation: lets us re-run with the same device-resident inputs
        self.fn = jax.jit(
            shard_map(
                _body,
                mesh=self.mesh,
                in_specs=in_specs,
                out_specs=out_specs,
                check_rep=False,
            ),
            keep_unused=True,
        )
        self.out_avals = out_avals

    def stage_inputs(self, in_maps):
        import jax
        from jax.sharding import NamedSharding, PartitionSpec

        sharding = NamedSharding(self.mesh, PartitionSpec("core"))
        args = []
        for name in self.in_names:
            concat = np.concatenate([np.asarray(m[name]) for m in in_maps], axis=0)
            args.append(jax.device_put(concat, sharding))
        for z in self.zero_outs:
            zc = np.zeros((N_CORES * z.shape[0], *z.shape[1:]), z.dtype)
            args.append(jax.device_put(zc, sharding))
        self.args = args

    def run(self):
        outs = self.fn(*self.args)
        self.jax.block_until_ready(outs)
        return outs

    def outputs_to_numpy(self, outs):
        per_core = []
        for c in range(N_CORES):
            per_core.append(
                {
                    name: np.asarray(outs[i]).reshape(
                        N_CORES, *self.out_avals[i].shape
                    )[c]
                    for i, name in enumerate(self.out_names)
                }
            )
        return per_core


_RUNNER_CACHE = {}


def get_runner(tok=TOK_SH):
    if tok not in _RUNNER_CACHE:
        _RUNNER_CACHE[tok] = PjrtRunner(_get_nc(tok))
    return _RUNNER_CACHE[tok]


def _kernel_np_fallback(x, qweight, qzeros, scales, g_idx, bias):
    shifts = (np.arange(8, dtype=np.int64) * 4)[None, :, None]
    wq = ((qweight.astype(np.int64)[:, None, :] >> shifts) & 0xF).reshape(
        IN_F, qweight.shape[1]
    )
    zq = (
        (qzeros.astype(np.int64)[:, :, None] >> shifts.reshape(1, 1, 8)) & 0xF
    ).reshape(qzeros.shape[0], -1) + 1
    w = scales[g_idx] * (wq.astype(np.float32) - zq[g_idx].astype(np.float32))
    return (x.astype(np.float32) @ w + bias).astype(np.float32)


def kernel(x, qweight, qzeros, scales, g_idx, bias):
    x = np.asarray(x)
    qweight = np.asarray(qweight)
    qzeros = np.asarray(qzeros)
    scales = np.asarray(scales)
    g_idx = np.asarray(g_idx)
    bias = np.asarray(bias)

    if not np.array_equal(
        g_idx, (np.arange(IN_F, dtype=np.int64) // GROUPSIZE).astype(g_idx.dtype)
    ):
        return _kernel_np_fallback(x, qweight, qzeros, scales, g_idx, bias)

    runner = get_runner()
    runner.stage_inputs(_shard_inputs(x, qweight, qzeros, scales, bias))
    outs = runner.run()
    return _assemble(runner.outputs_to_numpy(outs))


# revision 60
# speedup vs baseline: 1.0316x; 1.0316x over previous
"""GPTQ int4 quant linear: y = x @ dequant(qweight) + bias on 8 TRN2 cores.

Sharding: 2-way over tokens x 4-way over out_features (core c = (ti, oj)).
Host pre-transposes x to xT [in_f, tok] in bf16 with k rows permuted into
nibble-unpack order (pure layout prep), so the device kernel needs NO PE
transposes: every tensor-engine instruction is a real N=512 matmul (2048
per core, the floor given the 512-f32 PSUM bank limit; measured ~505us
per core, PE busy ~449us vs a 437us bf16 streaming floor). Scales ship
as bf16 so broadcast expands read the input DRAM tensor directly.

Dequant (all DVE, ~2.2us/k-tile): one i32 shift+mask extracts nibble
planes j and j+4 into the low/high u16 lanes at once, then per plane a
tensor_tensor subtract reads the strided u16 lanes directly (int->bf16
conversion inside the op) against the broadcast (z+1) tile, and a packed
bf16 multiply applies the scale. 32 dequantized k-tiles stay resident in
SBUF. The t=0 group/partition broadcasts ride two tiny PE matmuls with a
0/1 selector (DMA completion semaphores lag ~5-10us and would gate
W[0]); later groups use DMA expands off the critical path. Input DMAs
are spread across the sync/scalar/gpsimd queues for the same reason.

Main loop: token blocks sized [4]*7 + [2,2] (4 x 2 PSUM banks in
flight; the small tail blocks shrink the final drain latency). Per
block, k-major: stream xT chunks [128k, blk*128tok] (prefetch distance
4) and accumulate into the PSUM tiles; drain = bias add on DVE, out-DMA
on alternating scalar/gpsimd queues. Host assembles the 2x4 output grid.
"""

import numpy as np
import ml_dtypes

import concourse.bass as bass
import concourse.mybir as mybir
import concourse.tile as tile
from concourse import bacc

F32 = mybir.dt.float32
I32 = mybir.dt.int32
U16 = mybir.dt.uint16
BF16 = mybir.dt.bfloat16

N_CORES = 8
N_TOK_SHARDS = 2
N_OUT_SHARDS = 4
TOK = 8192
IN_F = 4096
OUT_F = 4096
TOK_SH = TOK // N_TOK_SHARDS  # 4096
OUT_SH = OUT_F // N_OUT_SHARDS  # 1024
PACKED_K = IN_F // 8  # 512 packed rows
GROUPSIZE = 128
N_GROUPS = IN_F // GROUPSIZE  # 32
P = 128
BLOCK_SIZES = [4] * 7 + [2, 2]  # token tiles per block
PF = 4  # x-chunk prefetch distance

ALU = mybir.AluOpType


def build_nc(tok=TOK_SH):
    n_mtiles = tok // P  # 32
    assert sum(BLOCK_SIZES) == n_mtiles
    n_t = PACKED_K // P  # 4 packed-row tiles
    n_kt = n_t * 8  # 32 k-tiles
    blk_off = np.cumsum([0] + BLOCK_SIZES)
    nc = bacc.Bacc(None, target_bir_lowering=False)

    # x arrives host-transposed (k in nibble-unpack order) and bf16
    xT = nc.dram_tensor("x", [IN_F, tok], BF16, kind="ExternalInput")
    qw = nc.dram_tensor("qw", [PACKED_K, OUT_SH], I32, kind="ExternalInput")
    qz = nc.dram_tensor("qz", [N_GROUPS, OUT_SH // 8], I32, kind="ExternalInput")
    sc = nc.dram_tensor("sc", [N_GROUPS, OUT_SH], BF16, kind="ExternalInput")
    bi = nc.dram_tensor("bi", [1, OUT_SH], F32, kind="ExternalInput")
    out = nc.dram_tensor("out", [tok, OUT_SH], F32, kind="ExternalOutput")

    with tile.TileContext(nc) as tc:
        with (
            tc.tile_pool(name="singles", bufs=1) as singles,
            tc.tile_pool(name="weights", bufs=1) as wpool,
            tc.tile_pool(name="qwin", bufs=1) as qwpool,
            tc.tile_pool(name="nib", bufs=4) as nibpool,
            tc.tile_pool(name="scexp", bufs=2) as scpool,
            tc.tile_pool(name="xin", bufs=PF + 2) as xpool,
            tc.tile_pool(name="yout", bufs=4) as ypool,
            tc.tile_pool(name="psum_y", bufs=4, space="PSUM") as psum_y,
        ):
            # ---- input DMAs, spread across queues (single-queue DMA
            # bandwidth + completion-semaphore lag would serialize the
            # W[0] critical path otherwise); qz first ----
            qz_sb = singles.tile([N_GROUPS, OUT_SH // 8], I32)
            nc.gpsimd.dma_start(qz_sb, qz[:, :])

            # 0/1 selector for the group->partition broadcast done on the
            # PE: exp_t[p, n] = sum_g E[g, 128t+p] * src[g, n] replicates
            # group row 8t + p//16 across the 16 partitions that use it.
            e_np = np.zeros((N_GROUPS, n_t * P), dtype=np.float32)
            for t in range(n_t):
                for p in range(P):
                    e_np[8 * t + p // 16, t * P + p] = 1.0
            e_dram = nc.inline_tensor(
                e_np.astype(ml_dtypes.bfloat16), name="gsel"
            )
            e_sb = singles.tile([N_GROUPS, n_t * P], BF16)
            nc.gpsimd.dma_start(e_sb, e_dram[:, :])

            qw_engines = [nc.scalar, nc.scalar, nc.gpsimd, nc.gpsimd]
            qw_tiles = []
            for t in range(n_t):
                qw_t = qwpool.tile([P, OUT_SH], I32, tag=f"qw{t}")
                qw_engines[t].dma_start(qw_t, qw[t * P : (t + 1) * P, :])
                qw_tiles.append(qw_t)

            bias_sb = singles.tile([P, OUT_SH], F32)
            nc.scalar.dma_start(out=bias_sb, in_=bi[:, :].to_broadcast((P, OUT_SH)))

            sc_sb = singles.tile([N_GROUPS, OUT_SH], BF16)
            nc.sync.dma_start(sc_sb, sc[:, :])

            # x chunk loads: chunk (b, kt) = xT[128 k, blk*128 tok]
            x_tiles = {}

            def load_chunk(b, kt):
                bs = BLOCK_SIZES[b]
                t0 = int(blk_off[b]) * P
                x_t = xpool.tile([P, bs * P], BF16, tag="x")
                nc.sync.dma_start(
                    x_t, xT[kt * P : (kt + 1) * P, t0 : t0 + bs * P]
                )
                x_tiles[(b, kt)] = x_t

            order = [(b, kt) for b in range(len(BLOCK_SIZES)) for kt in range(n_kt)]
            for i in range(PF):
                load_chunk(*order[i])

            # ---- zero-point prep: zq1[g, n] = (qz nibbles) + 1, bf16 ----
            zq1_i = singles.tile([N_GROUPS, OUT_SH], I32)
            zq1_i_r = zq1_i.rearrange("g (m j) -> g m j", j=8)
            for j in range(8):
                nc.vector.tensor_scalar(
                    out=zq1_i_r[:, :, j],
                    in0=qz_sb[:, :],
                    scalar1=4 * j,
                    scalar2=0xF,
                    op0=ALU.logical_shift_right,
                    op1=ALU.bitwise_and,
                )
            zq1_bf = singles.tile([N_GROUPS, OUT_SH], BF16)
            nc.vector.tensor_scalar(
                out=zq1_bf, in0=zq1_i, scalar1=1, scalar2=None, op0=ALU.add
            )

            # ---- dequantize weight shard into 32 resident bf16 tiles ----
            # tile (t, j) holds W rows k = 8*kk + j, kk in [128t, 128t+128);
            # partition kk maps to group 8t + (kk % 128)//16, hence the
            # 8-groups x 16-reps broadcast expansion.
            w_views = [None] * n_kt
            for t in range(n_t):
                zq1_exp = scpool.tile([P, OUT_SH], BF16, tag="zq1_exp")
                scale_exp = scpool.tile([P, OUT_SH], BF16, tag="scale_exp")
                if t == 0:
                    # t0 is on the W[0] critical path: broadcast via two
                    # tiny PE matmuls + a scalar copy (~3us) instead of
                    # DMA expands (~10us of completion-semaphore lag).
                    for src, dst in ((zq1_bf, zq1_exp), (sc_sb, scale_exp)):
                        ep = psum_y.tile([P, OUT_SH], F32, tag="y")
                        for h in range(2):
                            nc.tensor.matmul(
                                ep[:, h * 512 : (h + 1) * 512],
                                lhsT=e_sb[:, t * P : (t + 1) * P],
                                rhs=src[:, h * 512 : (h + 1) * 512],
                            )
                        nc.scalar.copy(dst, ep)
                else:
                    # off the critical path: DMA broadcast expands
                    # (SBUF->SBUF for zq1, straight from DRAM for scales)
                    nc.gpsimd.dma_start(
                        out=zq1_exp,
                        in_=bass.AP(
                            tensor=zq1_bf.tensor,
                            offset=zq1_bf.offset + t * 8 * OUT_SH,
                            ap=[[OUT_SH, 8], [0, 16], [1, OUT_SH]],
                        ),
                    )
                    nc.scalar.dma_start(
                        out=scale_exp,
                        in_=bass.AP(
                            tensor=sc,
                            offset=t * 8 * OUT_SH,
                            ap=[[OUT_SH, 8], [0, 16], [1, OUT_SH]],
                        ),
                    )
                qw_t = qw_tiles[t]
                for jj in range(4):
                    # Dual-plane extract: nibbles jj and jj+4 sit 16 bits
                    # apart, so one i32 shift+mask yields both planes in the
                    # low/high u16 lanes.
                    nib2 = nibpool.tile([P, OUT_SH], I32, tag="nib")
                    nc.vector.tensor_scalar(
                        out=nib2,
                        in0=qw_t,
                        scalar1=4 * jj,
                        scalar2=0x000F000F,
                        op0=ALU.logical_shift_right,
                        op1=ALU.bitwise_and,
                    )
                    nib2u = nib2[:, :].bitcast(U16).rearrange(
                        "p (n two) -> p n two", two=2
                    )
                    for half in range(2):
                        kt = t * 8 + jj + 4 * half
                        d = nibpool.tile([P, OUT_SH], BF16, tag="d")
                        nc.vector.tensor_tensor(
                            out=d,
                            in0=nib2u[:, :, half],
                            in1=zq1_exp,
                            op=ALU.subtract,
                        )
                        w = wpool.tile([P, OUT_SH], BF16, tag=f"w{kt}")
                        nc.vector.tensor_tensor(
                            out=w, in0=d, in1=scale_exp, op=ALU.mult
                        )
                        w_views[kt] = [
                            w[:, h * 512 : (h + 1) * 512] for h in range(2)
                        ]

            # ---- main loop: token blocks, k-major inside ----
            ndma = 0
            for b, bs in enumerate(BLOCK_SIZES):
                yps = []
                for i in range(bs):
                    yp = psum_y.tile([P, OUT_SH], F32, tag="y")
                    yps.append(yp)
                for kt in range(n_kt):
                    pos = b * n_kt + kt
                    if pos + PF < len(order):
                        load_chunk(*order[pos + PF])
                    xt = x_tiles.pop((b, kt))
                    for i in range(bs):
                        for h in range(2):
                            nc.tensor.matmul(
                                yps[i][:, h * 512 : (h + 1) * 512],
                                lhsT=xt[:, i * P : (i + 1) * P],
                                rhs=w_views[kt][h],
                                start=(kt == 0),
                                stop=(kt == n_kt - 1),
                            )
                for i in range(bs):
                    mi = int(blk_off[b]) + i
                    y_sb = ypool.tile([P, OUT_SH], F32, tag="y_sb")
                    nc.vector.tensor_add(y_sb, yps[i], bias_sb)
                    eng = nc.scalar if ndma % 2 == 0 else nc.gpsimd
                    eng.dma_start(out[mi * P : (mi + 1) * P, :], y_sb)
                    ndma += 1

    nc.compile()
    return nc


_NC_CACHE = {}


def _get_nc(tok=TOK_SH):
    if tok not in _NC_CACHE:
        _NC_CACHE[tok] = build_nc(tok)
    return _NC_CACHE[tok]


def _shard_inputs(x, qweight, qzeros, scales, bias, tok_sh=TOK_SH):
    # Device W tile (t, j) row r holds original k = 1024t + 8r + j (nibble
    # unpack order), i.e. device row d = 1024t + 128j + r. Permute x's k
    # axis to match while transposing to [k, tok] bf16.
    ntok = x.shape[0]
    xT = np.ascontiguousarray(
        np.asarray(x, dtype=np.float32)
        .reshape(ntok, 4, 128, 8)
        .transpose(1, 3, 2, 0)
        .reshape(IN_F, ntok)
        .astype(ml_dtypes.bfloat16)
    )
    sc_bf = np.asarray(scales, dtype=np.float32).astype(ml_dtypes.bfloat16)
    in_maps = []
    for c in range(N_CORES):
        ti, oj = divmod(c, N_OUT_SHARDS)
        sl = slice(oj * OUT_SH, (oj + 1) * OUT_SH)
        slz = slice(oj * (OUT_SH // 8), (oj + 1) * (OUT_SH // 8))
        in_maps.append(
            {
                "x": np.ascontiguousarray(
                    xT[:, ti * tok_sh : (ti + 1) * tok_sh]
                ),
                "qw": np.ascontiguousarray(qweight[:, sl], dtype=np.int32),
                "qz": np.ascontiguousarray(qzeros[:, slz], dtype=np.int32),
                "sc": np.ascontiguousarray(sc_bf[:, sl]),
                "bi": np.ascontiguousarray(
                    bias[sl].reshape(1, OUT_SH), dtype=np.float32
                ),
            }
        )
    return in_maps


def _assemble(per_core, tok_sh=TOK_SH):
    out = np.empty((N_TOK_SHARDS * tok_sh, OUT_F), dtype=np.float32)
    for c in range(N_CORES):
        ti, oj = divmod(c, N_OUT_SHARDS)
        out[ti * tok_sh : (ti + 1) * tok_sh, oj * OUT_SH : (oj + 1) * OUT_SH] = (
            per_core[c]["out"]
        )
    return out


class PjrtRunner:
    """Builds the shard_map'd bass executable once; supports timed re-runs."""

    def __init__(self, nc):
        import jax
        from jax.sharding import Mesh, PartitionSpec
        from jax.experimental.shard_map import shard_map
        from concourse import bass2jax, mybir as mb

        self.jax = jax
        bass2jax.install_neuronx_cc_hook()

        partition_name = (
            nc.partition_id_tensor.name if nc.partition_id_tensor else None
        )
        in_names, out_names, out_avals, zero_outs = [], [], [], []
        for alloc in nc.m.functions[0].allocations:
            if not isinstance(alloc, mb.MemoryLocationSet):
                continue
            name = alloc.memorylocations[0].name
            if alloc.kind == "ExternalInput":
                if name != partition_name:
                    in_names.append(name)
            elif alloc.kind == "ExternalOutput":
                shape = tuple(alloc.tensor_shape)
                dtype = mb.dt.np(alloc.dtype)
                out_names.append(name)
                out_avals.append(jax.core.ShapedArray(shape, dtype))
                zero_outs.append(np.zeros(shape, dtype))
        self.in_names = in_names
        self.out_names = out_names
        self.zero_outs = zero_outs
        n_params = len(in_names)
        all_in_names = in_names + out_names
        if partition_name is not None:
            all_in_names.append(partition_name)

        def _body(*args):
            operands = list(args)
            if partition_name is not None:
                operands.append(bass2jax.partition_id_tensor())
            outs = bass2jax._bass_exec_p.bind(
                *operands,
                out_avals=tuple(out_avals),
                in_names=tuple(all_in_names),
                out_names=tuple(out_names),
                lowering_input_output_aliases=(),
                sim_require_finite=True,
                sim_require_nnan=True,
                nc=nc,
            )
            return tuple(outs)

        devices = jax.devices()[:N_CORES]
        self.mesh = Mesh(np.asarray(devices), ("core",))
        in_specs = (PartitionSpec("core"),) * (n_params + len(out_names))
        out_specs = (PartitionSpec("core"),) * len(out_names)
        # no donation: lets us re-run with the same device-resident inputs
        self.fn = jax.jit(
            shard_map(
                _body,
                mesh=self.mesh,
                in_specs=in_specs,
                out_specs=out_specs,
                check_rep=False,
            ),
            keep_unused=True,
        )
        self.out_avals = out_avals

    def stage_inputs(self, in_maps):
        import jax
        from jax.sharding import NamedSharding, PartitionSpec

        sharding = NamedSharding(self.mesh, PartitionSpec("core"))
        args = []
        for name in self.in_names:
            concat = np.concatenate([np.asarray(m[name]) for m in in_maps], axis=0)
            args.append(jax.device_put(concat, sharding))
        for z in self.zero_outs:
            zc = np.zeros((N_CORES * z.shape[0], *z.shape[1:]), z.dtype)
            args.append(jax.device_put(zc, sharding))
        self.args = args

    def run(self):
        outs = self.fn(*self.args)
        self.jax.block_until_ready(outs)
        return outs

    def outputs_to_numpy(self, outs):
        per_core = []
        for c in range(N_CORES):
            per_core.append(
                {
                    name: np.asarray(outs[i]).reshape(
                        N_CORES, *self.out_avals[i].shape
                    )[c]
                    for i, name in enumerate(self.out_names)
                }
            )
        return per_core


_RUNNER_CACHE = {}


def get_runner(tok=TOK_SH):
    if tok not in _RUNNER_CACHE:
        _RUNNER_CACHE[tok] = PjrtRunner(_get_nc(tok))
    return _RUNNER_CACHE[tok]


def _kernel_np_fallback(x, qweight, qzeros, scales, g_idx, bias):
    shifts = (np.arange(8, dtype=np.int64) * 4)[None, :, None]
    wq = ((qweight.astype(np.int64)[:, None, :] >> shifts) & 0xF).reshape(
        IN_F, qweight.shape[1]
    )
    zq = (
        (qzeros.astype(np.int64)[:, :, None] >> shifts.reshape(1, 1, 8)) & 0xF
    ).reshape(qzeros.shape[0], -1) + 1
    w = scales[g_idx] * (wq.astype(np.float32) - zq[g_idx].astype(np.float32))
    return (x.astype(np.float32) @ w + bias).astype(np.float32)


def kernel(x, qweight, qzeros, scales, g_idx, bias):
    x = np.asarray(x)
    qweight = np.asarray(qweight)
    qzeros = np.asarray(qzeros)
    scales = np.asarray(scales)
    g_idx = np.asarray(g_idx)
    bias = np.asarray(bias)

    if not np.array_equal(
        g_idx, (np.arange(IN_F, dtype=np.int64) // GROUPSIZE).astype(g_idx.dtype)
    ):
        return _kernel_np_fallback(x, qweight, qzeros, scales, g_idx, bias)

    runner = get_runner()
    runner.stage_inputs(_shard_inputs(x, qweight, qzeros, scales, bias))
    outs = runner.run()
    return _assemble(runner.outputs_to_numpy(outs))


# revision 63
# speedup vs baseline: 1.2257x; 1.1882x over previous
"""GPTQ int4 quant linear: y = x @ dequant(qweight) + bias on 8 TRN2 cores.

Sharding: 2-way over tokens x 4-way over out_features (core c = (ti, oj)).
Host pre-transposes x to xT [in_f, tok] in bf16 with k rows permuted into
nibble-unpack order (pure layout prep), so the device kernel needs NO PE
transposes: every tensor-engine instruction is a real N=512 matmul (2048
per core, the floor given the 512-f32 PSUM bank limit; measured ~505us
per core, PE busy ~449us vs a 437us bf16 streaming floor). Scales ship
as bf16 so broadcast expands read the input DRAM tensor directly.

Dequant (all DVE, ~2.2us/k-tile): one i32 shift+mask extracts nibble
planes j and j+4 into the low/high u16 lanes at once, then per plane a
tensor_tensor subtract reads the strided u16 lanes directly (int->bf16
conversion inside the op) against the broadcast (z+1) tile, and a packed
bf16 multiply applies the scale. 32 dequantized k-tiles stay resident in
SBUF. The t=0 group/partition broadcasts ride two tiny PE matmuls with a
0/1 selector (DMA completion semaphores lag ~5-10us and would gate
W[0]); later groups use DMA expands off the critical path. Input DMAs
are spread across the sync/scalar/gpsimd queues for the same reason.

Main loop: token blocks sized [4]*7 + [2,2] (4 x 2 PSUM banks in
flight; the small tail blocks shrink the final drain latency). Per
block, k-major: stream xT chunks [128k, blk*128tok] (prefetch distance
4) and accumulate into the PSUM tiles; drain = bias add on DVE, out-DMA
on alternating scalar/gpsimd queues. Host assembles the 2x4 output grid.
"""

import numpy as np
import ml_dtypes

import concourse.bass as bass
import concourse.mybir as mybir
import concourse.tile as tile
from concourse import bacc

F32 = mybir.dt.float32
I32 = mybir.dt.int32
U16 = mybir.dt.uint16
BF16 = mybir.dt.bfloat16

N_CORES = 8
N_TOK_SHARDS = 2
N_OUT_SHARDS = 4
TOK = 8192
IN_F = 4096
OUT_F = 4096
TOK_SH = TOK // N_TOK_SHARDS  # 4096
OUT_SH = OUT_F // N_OUT_SHARDS  # 1024
PACKED_K = IN_F // 8  # 512 packed rows
GROUPSIZE = 128
N_GROUPS = IN_F // GROUPSIZE  # 32
P = 128
BLOCK_SIZES = [4] * 7 + [2, 2]  # token tiles per block
PF = 5  # x-chunk prefetch distance

ALU = mybir.AluOpType


def build_nc(tok=TOK_SH):
    n_mtiles = tok // P  # 32
    assert sum(BLOCK_SIZES) == n_mtiles
    n_t = PACKED_K // P  # 4 packed-row tiles
    n_kt = n_t * 8  # 32 k-tiles
    blk_off = np.cumsum([0] + BLOCK_SIZES)
    nc = bacc.Bacc(None, target_bir_lowering=False)

    # x arrives host-transposed (k in nibble-unpack order) and bf16
    xT = nc.dram_tensor("x", [IN_F, tok], BF16, kind="ExternalInput")
    qw = nc.dram_tensor("qw", [PACKED_K, OUT_SH], I32, kind="ExternalInput")
    qz = nc.dram_tensor("qz", [N_GROUPS, OUT_SH // 8], I32, kind="ExternalInput")
    sc = nc.dram_tensor("sc", [N_GROUPS, OUT_SH], BF16, kind="ExternalInput")
    bi = nc.dram_tensor("bi", [1, OUT_SH], F32, kind="ExternalInput")
    out = nc.dram_tensor("out", [tok, OUT_SH], F32, kind="ExternalOutput")

    with tile.TileContext(nc) as tc:
        with (
            tc.tile_pool(name="singles", bufs=1) as singles,
            tc.tile_pool(name="weights", bufs=1) as wpool,
            tc.tile_pool(name="qwin", bufs=1) as qwpool,
            tc.tile_pool(name="nib", bufs=4) as nibpool,
            tc.tile_pool(name="scexp", bufs=2) as scpool,
            tc.tile_pool(name="xin", bufs=PF + 2) as xpool,
            tc.tile_pool(name="yout", bufs=4) as ypool,
            tc.tile_pool(name="psum_y", bufs=4, space="PSUM") as psum_y,
        ):
            # ---- input DMAs, spread across queues (single-queue DMA
            # bandwidth + completion-semaphore lag would serialize the
            # W[0] critical path otherwise); qz first ----
            qz_sb = singles.tile([N_GROUPS, OUT_SH // 8], I32)
            nc.gpsimd.dma_start(qz_sb, qz[:, :])

            # 0/1 selector for the group->partition broadcast done on the
            # PE: exp_t[p, n] = sum_g E[g, 128t+p] * src[g, n] replicates
            # group row 8t + p//16 across the 16 partitions that use it.
            e_np = np.zeros((N_GROUPS, n_t * P), dtype=np.float32)
            for t in range(n_t):
                for p in range(P):
                    e_np[8 * t + p // 16, t * P + p] = 1.0
            e_dram = nc.inline_tensor(
                e_np.astype(ml_dtypes.bfloat16), name="gsel"
            )
            e_sb = singles.tile([N_GROUPS, n_t * P], BF16)
            nc.gpsimd.dma_start(e_sb, e_dram[:, :])

            qw_engines = [nc.scalar, nc.scalar, nc.gpsimd, nc.gpsimd]
            qw_tiles = []
            for t in range(n_t):
                qw_t = qwpool.tile([P, OUT_SH], I32, tag=f"qw{t}")
                qw_engines[t].dma_start(qw_t, qw[t * P : (t + 1) * P, :])
                qw_tiles.append(qw_t)

            bias_sb = singles.tile([P, OUT_SH], F32)
            nc.scalar.dma_start(out=bias_sb, in_=bi[:, :].to_broadcast((P, OUT_SH)))

            sc_sb = singles.tile([N_GROUPS, OUT_SH], BF16)
            nc.sync.dma_start(sc_sb, sc[:, :])

            # x chunk loads: chunk (b, kt) = xT[128 k, blk*128 tok]
            x_tiles = {}

            def load_chunk(b, kt):
                bs = BLOCK_SIZES[b]
                t0 = int(blk_off[b]) * P
                # per-size tags: mixed tile sizes in one ring tag stall the
                # allocator at the 4-tile -> 2-tile block transition
                x_t = xpool.tile([P, bs * P], BF16, tag=f"x{bs}")
                nc.sync.dma_start(
                    x_t, xT[kt * P : (kt + 1) * P, t0 : t0 + bs * P]
                )
                x_tiles[(b, kt)] = x_t

            order = [(b, kt) for b in range(len(BLOCK_SIZES)) for kt in range(n_kt)]
            for i in range(PF):
                load_chunk(*order[i])

            # ---- zero-point prep: zq1[g, n] = (qz nibbles) + 1, bf16 ----
            zq1_i = singles.tile([N_GROUPS, OUT_SH], I32)
            zq1_i_r = zq1_i.rearrange("g (m j) -> g m j", j=8)
            for j in range(8):
                nc.vector.tensor_scalar(
                    out=zq1_i_r[:, :, j],
                    in0=qz_sb[:, :],
                    scalar1=4 * j,
                    scalar2=0xF,
                    op0=ALU.logical_shift_right,
                    op1=ALU.bitwise_and,
                )
            zq1_bf = singles.tile([N_GROUPS, OUT_SH], BF16)
            nc.vector.tensor_scalar(
                out=zq1_bf, in0=zq1_i, scalar1=1, scalar2=None, op0=ALU.add
            )

            # ---- dequantize weight shard into 32 resident bf16 tiles ----
            # tile (t, j) holds W rows k = 8*kk + j, kk in [128t, 128t+128);
            # partition kk maps to group 8t + (kk % 128)//16, hence the
            # 8-groups x 16-reps broadcast expansion.
            w_views = [None] * n_kt
            for t in range(n_t):
                zq1_exp = scpool.tile([P, OUT_SH], BF16, tag="zq1_exp")
                scale_exp = scpool.tile([P, OUT_SH], BF16, tag="scale_exp")
                if t == 0:
                    # t0 is on the W[0] critical path: broadcast via two
                    # tiny PE matmuls + a scalar copy (~3us) instead of
                    # DMA expands (~10us of completion-semaphore lag).
                    for src, dst in ((zq1_bf, zq1_exp), (sc_sb, scale_exp)):
                        ep = psum_y.tile([P, OUT_SH], F32, tag="y")
                        for h in range(2):
                            nc.tensor.matmul(
                                ep[:, h * 512 : (h + 1) * 512],
                                lhsT=e_sb[:, t * P : (t + 1) * P],
                                rhs=src[:, h * 512 : (h + 1) * 512],
                            )
                        nc.scalar.copy(dst, ep)
                else:
                    # off the critical path: DMA broadcast expands
                    # (SBUF->SBUF for zq1, straight from DRAM for scales)
                    nc.gpsimd.dma_start(
                        out=zq1_exp,
                        in_=bass.AP(
                            tensor=zq1_bf.tensor,
                            offset=zq1_bf.offset + t * 8 * OUT_SH,
                            ap=[[OUT_SH, 8], [0, 16], [1, OUT_SH]],
                        ),
                    )
                    nc.scalar.dma_start(
                        out=scale_exp,
                        in_=bass.AP(
                            tensor=sc,
                            offset=t * 8 * OUT_SH,
                            ap=[[OUT_SH, 8], [0, 16], [1, OUT_SH]],
                        ),
                    )
                qw_t = qw_tiles[t]
                for jj in range(4):
                    # Dual-plane extract: nibbles jj and jj+4 sit 16 bits
                    # apart, so one i32 shift+mask yields both planes in the
                    # low/high u16 lanes.
                    nib2 = nibpool.tile([P, OUT_SH], I32, tag="nib")
                    nc.vector.tensor_scalar(
                        out=nib2,
                        in0=qw_t,
                        scalar1=4 * jj,
                        scalar2=0x000F000F,
                        op0=ALU.logical_shift_right,
                        op1=ALU.bitwise_and,
                    )
                    nib2u = nib2[:, :].bitcast(U16).rearrange(
                        "p (n two) -> p n two", two=2
                    )
                    for half in range(2):
                        kt = t * 8 + jj + 4 * half
                        d = nibpool.tile([P, OUT_SH], BF16, tag="d")
                        nc.vector.tensor_tensor(
                            out=d,
                            in0=nib2u[:, :, half],
                            in1=zq1_exp,
                            op=ALU.subtract,
                        )
                        w = wpool.tile([P, OUT_SH], BF16, tag=f"w{kt}")
                        nc.vector.tensor_tensor(
                            out=w, in0=d, in1=scale_exp, op=ALU.mult
                        )
                        w_views[kt] = [
                            w[:, h * 512 : (h + 1) * 512] for h in range(2)
                        ]

            # ---- main loop: token blocks, k-major inside ----
            ndma = 0
            for b, bs in enumerate(BLOCK_SIZES):
                yps = []
                for i in range(bs):
                    yp = psum_y.tile([P, OUT_SH], F32, tag="y")
                    yps.append(yp)
                for kt in range(n_kt):
                    pos = b * n_kt + kt
                    if pos + PF < len(order):
                        load_chunk(*order[pos + PF])
                    xt = x_tiles.pop((b, kt))
                    # h outer: the first weight half alone feeds bs matmuls,
                    # halving the W-latency the first matmul of a k-tile sees
                    for h in range(2):
                        for i in range(bs):
                            nc.tensor.matmul(
                                yps[i][:, h * 512 : (h + 1) * 512],
                                lhsT=xt[:, i * P : (i + 1) * P],
                                rhs=w_views[kt][h],
                                start=(kt == 0),
                                stop=(kt == n_kt - 1),
                            )
                for i in range(bs):
                    mi = int(blk_off[b]) + i
                    y_sb = ypool.tile([P, OUT_SH], F32, tag="y_sb")
                    nc.vector.tensor_add(y_sb, yps[i], bias_sb)
                    eng = nc.scalar if ndma % 2 == 0 else nc.gpsimd
                    eng.dma_start(out[mi * P : (mi + 1) * P, :], y_sb)
                    ndma += 1

    nc.compile()
    return nc


_NC_CACHE = {}


def _get_nc(tok=TOK_SH):
    if tok not in _NC_CACHE:
        _NC_CACHE[tok] = build_nc(tok)
    return _NC_CACHE[tok]


def _shard_inputs(x, qweight, qzeros, scales, bias, tok_sh=TOK_SH):
    # Device W tile (t, j) row r holds original k = 1024t + 8r + j (nibble
    # unpack order), i.e. device row d = 1024t + 128j + r. Permute x's k
    # axis to match while transposing to [k, tok] bf16.
    ntok = x.shape[0]
    xT = np.ascontiguousarray(
        np.asarray(x, dtype=np.float32)
        .reshape(ntok, 4, 128, 8)
        .transpose(1, 3, 2, 0)
        .reshape(IN_F, ntok)
        .astype(ml_dtypes.bfloat16)
    )
    sc_bf = np.asarray(scales, dtype=np.float32).astype(ml_dtypes.bfloat16)
    in_maps = []
    for c in range(N_CORES):
        ti, oj = divmod(c, N_OUT_SHARDS)
        sl = slice(oj * OUT_SH, (oj + 1) * OUT_SH)
        slz = slice(oj * (OUT_SH // 8), (oj + 1) * (OUT_SH // 8))
        in_maps.append(
            {
                "x": np.ascontiguousarray(
                    xT[:, ti * tok_sh : (ti + 1) * tok_sh]
                ),
                "qw": np.ascontiguousarray(qweight[:, sl], dtype=np.int32),
                "qz": np.ascontiguousarray(qzeros[:, slz], dtype=np.int32),
                "sc": np.ascontiguousarray(sc_bf[:, sl]),
                "bi": np.ascontiguousarray(
                    bias[sl].reshape(1, OUT_SH), dtype=np.float32
                ),
            }
        )
    return in_maps


def _assemble(per_core, tok_sh=TOK_SH):
    out = np.empty((N_TOK_SHARDS * tok_sh, OUT_F), dtype=np.float32)
    for c in range(N_CORES):
        ti, oj = divmod(c, N_OUT_SHARDS)
        out[ti * tok_sh : (ti + 1) * tok_sh, oj * OUT_SH : (oj + 1) * OUT_SH] = (
            per_core[c]["out"]
        )
    return out


class PjrtRunner:
    """Builds the shard_map'd bass executable once; supports timed re-runs."""

    def __init__(self, nc):
        import jax
        from jax.sharding import Mesh, PartitionSpec
        from jax.experimental.shard_map import shard_map
        from concourse import bass2jax, mybir as mb

        self.jax = jax
        bass2jax.install_neuronx_cc_hook()

        partition_name = (
            nc.partition_id_tensor.name if nc.partition_id_tensor else None
        )
        in_names, out_names, out_avals, zero_outs = [], [], [], []
        for alloc in nc.m.functions[0].allocations:
            if not isinstance(alloc, mb.MemoryLocationSet):
                continue
            name = alloc.memorylocations[0].name
            if alloc.kind == "ExternalInput":
                if name != partition_name:
                    in_names.append(name)
            elif alloc.kind == "ExternalOutput":
                shape = tuple(alloc.tensor_shape)
                dtype = mb.dt.np(alloc.dtype)
                out_names.append(name)
                out_avals.append(jax.core.ShapedArray(shape, dtype))
                zero_outs.append(np.zeros(shape, dtype))
        self.in_names = in_names
        self.out_names = out_names
        self.zero_outs = zero_outs
        n_params = len(in_names)
        all_in_names = in_names + out_names
        if partition_name is not None:
            all_in_names.append(partition_name)

        def _body(*args):
            operands = list(args)
            if partition_name is not None:
                operands.append(bass2jax.partition_id_tensor())
            outs = bass2jax._bass_exec_p.bind(
                *operands,
                out_avals=tuple(out_avals),
                in_names=tuple(all_in_names),
                out_names=tuple(out_names),
                lowering_input_output_aliases=(),
                sim_require_finite=True,
                sim_require_nnan=True,
                nc=nc,
            )
            return tuple(outs)

        devices = jax.devices()[:N_CORES]
        self.mesh = Mesh(np.asarray(devices), ("core",))
        in_specs = (PartitionSpec("core"),) * (n_params + len(out_names))
        out_specs = (PartitionSpec("core"),) * len(out_names)
        # no donation: lets us re-run with the same device-resident inputs
        self.fn = jax.jit(
            shard_map(
                _body,
                mesh=self.mesh,
                in_specs=in_specs,
                out_specs=out_specs,
                check_rep=False,
            ),
            keep_unused=True,
        )
        self.out_avals = out_avals

    def stage_inputs(self, in_maps):
        import jax
        from jax.sharding import NamedSharding, PartitionSpec

        sharding = NamedSharding(self.mesh, PartitionSpec("core"))
        args = []
        for name in self.in_names:
            concat = np.concatenate([np.asarray(m[name]) for m in in_maps], axis=0)
            args.append(jax.device_put(concat, sharding))
        for z in self.zero_outs:
            zc = np.zeros((N_CORES * z.shape[0], *z.shape[1:]), z.dtype)
            args.append(jax.device_put(zc, sharding))
        self.args = args

    def run(self):
        outs = self.fn(*self.args)
        self.jax.block_until_ready(outs)
        return outs

    def outputs_to_numpy(self, outs):
        per_core = []
        for c in range(N_CORES):
            per_core.append(
                {
                    name: np.asarray(outs[i]).reshape(
                        N_CORES, *self.out_avals[i].shape
                    )[c]
                    for i, name in enumerate(self.out_names)
                }
            )
        return per_core


_RUNNER_CACHE = {}


def get_runner(tok=TOK_SH):
    if tok not in _RUNNER_CACHE:
        _RUNNER_CACHE[tok] = PjrtRunner(_get_nc(tok))
    return _RUNNER_CACHE[tok]


def _kernel_np_fallback(x, qweight, qzeros, scales, g_idx, bias):
    shifts = (np.arange(8, dtype=np.int64) * 4)[None, :, None]
    wq = ((qweight.astype(np.int64)[:, None, :] >> shifts) & 0xF).reshape(
        IN_F, qweight.shape[1]
    )
    zq = (
        (qzeros.astype(np.int64)[:, :, None] >> shifts.reshape(1, 1, 8)) & 0xF
    ).reshape(qzeros.shape[0], -1) + 1
    w = scales[g_idx] * (wq.astype(np.float32) - zq[g_idx].astype(np.float32))
    return (x.astype(np.float32) @ w + bias).astype(np.float32)


def kernel(x, qweight, qzeros, scales, g_idx, bias):
    x = np.asarray(x)
    qweight = np.asarray(qweight)
    qzeros = np.asarray(qzeros)
    scales = np.asarray(scales)
    g_idx = np.asarray(g_idx)
    bias = np.asarray(bias)

    if not np.array_equal(
        g_idx, (np.arange(IN_F, dtype=np.int64) // GROUPSIZE).astype(g_idx.dtype)
    ):
        return _kernel_np_fallback(x, qweight, qzeros, scales, g_idx, bias)

    runner = get_runner()
    runner.stage_inputs(_shard_inputs(x, qweight, qzeros, scales, bias))
    outs = runner.run()
    return _assemble(runner.outputs_to_numpy(outs))


# revision 64
# speedup vs baseline: 1.2333x; 1.0061x over previous
"""GPTQ int4 quant linear: y = x @ dequant(qweight) + bias on 8 TRN2 cores.

Sharding: 2-way over tokens x 4-way over out_features (core c = (ti, oj)).
Host pre-transposes x to xT [in_f, tok] in bf16 with k rows permuted into
nibble-unpack order (pure layout prep), so the device kernel needs NO PE
transposes: every tensor-engine instruction is a real N=512 matmul (2048
per core, the floor given the 512-f32 PSUM bank limit; measured ~505us
per core, PE busy ~449us vs a 437us bf16 streaming floor). Scales ship
as bf16 so broadcast expands read the input DRAM tensor directly.

Dequant (all DVE, ~2.2us/k-tile): one i32 shift+mask extracts nibble
planes j and j+4 into the low/high u16 lanes at once, then per plane a
tensor_tensor subtract reads the strided u16 lanes directly (int->bf16
conversion inside the op) against the broadcast (z+1) tile, and a packed
bf16 multiply applies the scale. 32 dequantized k-tiles stay resident in
SBUF. The t=0 group/partition broadcasts ride two tiny PE matmuls with a
0/1 selector (DMA completion semaphores lag ~5-10us and would gate
W[0]); later groups use DMA expands off the critical path. Input DMAs
are spread across the sync/scalar/gpsimd queues for the same reason.

Main loop: token blocks sized [4]*7 + [2,2] (4 x 2 PSUM banks in
flight; the small tail blocks shrink the final drain latency). Per
block, k-major: stream xT chunks [128k, blk*128tok] (prefetch distance
4) and accumulate into the PSUM tiles; drain = bias add on DVE, out-DMA
on alternating scalar/gpsimd queues. Host assembles the 2x4 output grid.
"""

import numpy as np
import ml_dtypes

import concourse.bass as bass
import concourse.mybir as mybir
import concourse.tile as tile
from concourse import bacc

F32 = mybir.dt.float32
I32 = mybir.dt.int32
U16 = mybir.dt.uint16
BF16 = mybir.dt.bfloat16

N_CORES = 8
N_TOK_SHARDS = 2
N_OUT_SHARDS = 4
TOK = 8192
IN_F = 4096
OUT_F = 4096
TOK_SH = TOK // N_TOK_SHARDS  # 4096
OUT_SH = OUT_F // N_OUT_SHARDS  # 1024
PACKED_K = IN_F // 8  # 512 packed rows
GROUPSIZE = 128
N_GROUPS = IN_F // GROUPSIZE  # 32
P = 128
BLOCK_SIZES = [4] * 7 + [2, 2]  # token tiles per block
PF = 5  # x-chunk prefetch distance

ALU = mybir.AluOpType


def build_nc(tok=TOK_SH):
    n_mtiles = tok // P  # 32
    assert sum(BLOCK_SIZES) == n_mtiles
    n_t = PACKED_K // P  # 4 packed-row tiles
    n_kt = n_t * 8  # 32 k-tiles
    blk_off = np.cumsum([0] + BLOCK_SIZES)
    nc = bacc.Bacc(None, target_bir_lowering=False)

    # x arrives host-transposed (k in nibble-unpack order) and bf16
    xT = nc.dram_tensor("x", [IN_F, tok], BF16, kind="ExternalInput")
    qw = nc.dram_tensor("qw", [PACKED_K, OUT_SH], I32, kind="ExternalInput")
    qz = nc.dram_tensor("qz", [N_GROUPS, OUT_SH // 8], I32, kind="ExternalInput")
    sc = nc.dram_tensor("sc", [N_GROUPS, OUT_SH], BF16, kind="ExternalInput")
    bi = nc.dram_tensor("bi", [1, OUT_SH], F32, kind="ExternalInput")
    out = nc.dram_tensor("out", [tok, OUT_SH], F32, kind="ExternalOutput")

    with tile.TileContext(nc) as tc:
        with (
            tc.tile_pool(name="singles", bufs=1) as singles,
            tc.tile_pool(name="weights", bufs=1) as wpool,
            tc.tile_pool(name="qwin", bufs=1) as qwpool,
            tc.tile_pool(name="nib", bufs=4) as nibpool,
            tc.tile_pool(name="scexp", bufs=2) as scpool,
            tc.tile_pool(name="xin", bufs=PF + 2) as xpool,
            tc.tile_pool(name="yout", bufs=4) as ypool,
            tc.tile_pool(name="psum_y", bufs=4, space="PSUM") as psum_y,
        ):
            # ---- input DMAs, spread across queues (single-queue DMA
            # bandwidth + completion-semaphore lag would serialize the
            # W[0] critical path otherwise); qz first ----
            qz_sb = singles.tile([N_GROUPS, OUT_SH // 8], I32)
            nc.gpsimd.dma_start(qz_sb, qz[:, :])

            # 0/1 selector for the group->partition broadcast done on the
            # PE: exp_t[p, n] = sum_g E[g, 128t+p] * src[g, n] replicates
            # group row 8t + p//16 across the 16 partitions that use it.
            e_np = np.zeros((N_GROUPS, n_t * P), dtype=np.float32)
            for t in range(n_t):
                for p in range(P):
                    e_np[8 * t + p // 16, t * P + p] = 1.0
            e_dram = nc.inline_tensor(
                e_np.astype(ml_dtypes.bfloat16), name="gsel"
            )
            e_sb = singles.tile([N_GROUPS, n_t * P], BF16)
            nc.gpsimd.dma_start(e_sb, e_dram[:, :])

            qw_engines = [nc.scalar, nc.scalar, nc.gpsimd, nc.gpsimd]
            qw_tiles = []
            for t in range(n_t):
                qw_t = qwpool.tile([P, OUT_SH], I32, tag=f"qw{t}")
                qw_engines[t].dma_start(qw_t, qw[t * P : (t + 1) * P, :])
                qw_tiles.append(qw_t)

            bias_sb = singles.tile([P, OUT_SH], F32)
            nc.scalar.dma_start(out=bias_sb, in_=bi[:, :].to_broadcast((P, OUT_SH)))

            sc_sb = singles.tile([N_GROUPS, OUT_SH], BF16)
            nc.sync.dma_start(sc_sb, sc[:, :])

            # x chunk loads: chunk (b, kt) = xT[128 k, blk*128 tok]
            x_tiles = {}

            def load_chunk(b, kt):
                bs = BLOCK_SIZES[b]
                t0 = int(blk_off[b]) * P
                # per-size tags: mixed tile sizes in one ring tag stall the
                # allocator at the 4-tile -> 2-tile block transition
                x_t = xpool.tile([P, bs * P], BF16, tag=f"x{bs}")
                nc.sync.dma_start(
                    x_t, xT[kt * P : (kt + 1) * P, t0 : t0 + bs * P]
                )
                x_tiles[(b, kt)] = x_t

            order = [(b, kt) for b in range(len(BLOCK_SIZES)) for kt in range(n_kt)]
            for i in range(PF):
                load_chunk(*order[i])

            # ---- zero-point prep: zq1[g, n] = (qz nibbles) + 1, bf16 ----
            zq1_i = singles.tile([N_GROUPS, OUT_SH], I32)
            zq1_i_r = zq1_i.rearrange("g (m j) -> g m j", j=8)
            for j in range(8):
                nc.vector.tensor_scalar(
                    out=zq1_i_r[:, :, j],
                    in0=qz_sb[:, :],
                    scalar1=4 * j,
                    scalar2=0xF,
                    op0=ALU.logical_shift_right,
                    op1=ALU.bitwise_and,
                )
            zq1_bf = singles.tile([N_GROUPS, OUT_SH], BF16)
            nc.vector.tensor_scalar(
                out=zq1_bf, in0=zq1_i, scalar1=1, scalar2=None, op0=ALU.add
            )

            # ---- dequantize weight shard into 32 resident bf16 tiles ----
            # tile (t, j) holds W rows k = 8*kk + j, kk in [128t, 128t+128);
            # partition kk maps to group 8t + (kk % 128)//16, hence the
            # 8-groups x 16-reps broadcast expansion.
            w_views = [None] * n_kt
            for t in range(n_t):
                zq1_exp = scpool.tile([P, OUT_SH], BF16, tag="zq1_exp")
                scale_exp = scpool.tile([P, OUT_SH], BF16, tag="scale_exp")
                if t == 0:
                    # t0 is on the W[0] critical path: broadcast via two
                    # tiny PE matmuls + a scalar copy (~3us) instead of
                    # DMA expands (~10us of completion-semaphore lag).
                    for src, dst in ((zq1_bf, zq1_exp), (sc_sb, scale_exp)):
                        ep = psum_y.tile([P, OUT_SH], F32, tag="y")
                        for h in range(2):
                            nc.tensor.matmul(
                                ep[:, h * 512 : (h + 1) * 512],
                                lhsT=e_sb[:, t * P : (t + 1) * P],
                                rhs=src[:, h * 512 : (h + 1) * 512],
                            )
                        nc.scalar.copy(dst, ep)
                else:
                    # off the critical path: DMA broadcast expands
                    # (SBUF->SBUF for zq1, straight from DRAM for scales)
                    nc.gpsimd.dma_start(
                        out=zq1_exp,
                        in_=bass.AP(
                            tensor=zq1_bf.tensor,
                            offset=zq1_bf.offset + t * 8 * OUT_SH,
                            ap=[[OUT_SH, 8], [0, 16], [1, OUT_SH]],
                        ),
                    )
                    nc.scalar.dma_start(
                        out=scale_exp,
                        in_=bass.AP(
                            tensor=sc,
                            offset=t * 8 * OUT_SH,
                            ap=[[OUT_SH, 8], [0, 16], [1, OUT_SH]],
                        ),
                    )
                qw_t = qw_tiles[t]
                for jj in range(4):
                    # Dual-plane extract: nibbles jj and jj+4 sit 16 bits
                    # apart, so one i32 shift+mask yields both planes in the
                    # low/high u16 lanes.
                    nib2 = nibpool.tile([P, OUT_SH], I32, tag="nib")
                    nc.vector.tensor_scalar(
                        out=nib2,
                        in0=qw_t,
                        scalar1=4 * jj,
                        scalar2=0x000F000F,
                        op0=ALU.logical_shift_right,
                        op1=ALU.bitwise_and,
                    )
                    nib2u = nib2[:, :].bitcast(U16).rearrange(
                        "p (n two) -> p n two", two=2
                    )
                    for half in range(2):
                        kt = t * 8 + jj + 4 * half
                        d = nibpool.tile([P, OUT_SH], BF16, tag="d")
                        nc.vector.tensor_tensor(
                            out=d,
                            in0=nib2u[:, :, half],
                            in1=zq1_exp,
                            op=ALU.subtract,
                        )
                        w = wpool.tile([P, OUT_SH], BF16, tag=f"w{kt}")
                        nc.vector.tensor_tensor(
                            out=w, in0=d, in1=scale_exp, op=ALU.mult
                        )
                        w_views[kt] = [
                            w[:, h * 512 : (h + 1) * 512] for h in range(2)
                        ]

            # ---- main loop: token blocks, k-major inside ----
            ndma = 0
            for b, bs in enumerate(BLOCK_SIZES):
                yps = []
                for i in range(bs):
                    yp = psum_y.tile([P, OUT_SH], F32, tag="y")
                    yps.append(yp)
                for kt in range(n_kt):
                    pos = b * n_kt + kt
                    if pos + PF < len(order):
                        load_chunk(*order[pos + PF])
                    xt = x_tiles.pop((b, kt))
                    # i outer: tile i's last matmul lands 2*(bs-i) slots
                    # before the block boundary, giving its drain time to
                    # free the PSUM slot before the next block claims it
                    for i in range(bs):
                        for h in range(2):
                            nc.tensor.matmul(
                                yps[i][:, h * 512 : (h + 1) * 512],
                                lhsT=xt[:, i * P : (i + 1) * P],
                                rhs=w_views[kt][h],
                                start=(kt == 0),
                                stop=(kt == n_kt - 1),
                            )
                for i in range(bs):
                    mi = int(blk_off[b]) + i
                    y_sb = ypool.tile([P, OUT_SH], F32, tag="y_sb")
                    nc.vector.tensor_add(y_sb, yps[i], bias_sb)
                    eng = nc.scalar if ndma % 2 == 0 else nc.gpsimd
                    eng.dma_start(out[mi * P : (mi + 1) * P, :], y_sb)
                    ndma += 1

    nc.compile()
    return nc


_NC_CACHE = {}


def _get_nc(tok=TOK_SH):
    if tok not in _NC_CACHE:
        _NC_CACHE[tok] = build_nc(tok)
    return _NC_CACHE[tok]


def _shard_inputs(x, qweight, qzeros, scales, bias, tok_sh=TOK_SH):
    # Device W tile (t, j) row r holds original k = 1024t + 8r + j (nibble
    # unpack order), i.e. device row d = 1024t + 128j + r. Permute x's k
    # axis to match while transposing to [k, tok] bf16.
    ntok = x.shape[0]
    xT = np.ascontiguousarray(
        np.asarray(x, dtype=np.float32)
        .reshape(ntok, 4, 128, 8)
        .transpose(1, 3, 2, 0)
        .reshape(IN_F, ntok)
        .astype(ml_dtypes.bfloat16)
    )
    sc_bf = np.asarray(scales, dtype=np.float32).astype(ml_dtypes.bfloat16)
    in_maps = []
    for c in range(N_CORES):
        ti, oj = divmod(c, N_OUT_SHARDS)
        sl = slice(oj * OUT_SH, (oj + 1) * OUT_SH)
        slz = slice(oj * (OUT_SH // 8), (oj + 1) * (OUT_SH // 8))
        in_maps.append(
            {
                "x": np.ascontiguousarray(
                    xT[:, ti * tok_sh : (ti + 1) * tok_sh]
                ),
                "qw": np.ascontiguousarray(qweight[:, sl], dtype=np.int32),
                "qz": np.ascontiguousarray(qzeros[:, slz], dtype=np.int32),
                "sc": np.ascontiguousarray(sc_bf[:, sl]),
                "bi": np.ascontiguousarray(
                    bias[sl].reshape(1, OUT_SH), dtype=np.float32
                ),
            }
        )
    return in_maps


def _assemble(per_core, tok_sh=TOK_SH):
    out = np.empty((N_TOK_SHARDS * tok_sh, OUT_F), dtype=np.float32)
    for c in range(N_CORES):
        ti, oj = divmod(c, N_OUT_SHARDS)
        out[ti * tok_sh : (ti + 1) * tok_sh, oj * OUT_SH : (oj + 1) * OUT_SH] = (
            per_core[c]["out"]
        )
    return out


class PjrtRunner:
    """Builds the shard_map'd bass executable once; supports timed re-runs."""

    def __init__(self, nc):
        import jax
        from jax.sharding import Mesh, PartitionSpec
        from jax.experimental.shard_map import shard_map
        from concourse import bass2jax, mybir as mb

        self.jax = jax
        bass2jax.install_neuronx_cc_hook()

        partition_name = (
            nc.partition_id_tensor.name if nc.partition_id_tensor else None
        )
        in_names, out_names, out_avals, zero_outs = [], [], [], []
        for alloc in nc.m.functions[0].allocations:
            if not isinstance(alloc, mb.MemoryLocationSet):
                continue
            name = alloc.memorylocations[0].name
            if alloc.kind == "ExternalInput":
                if name != partition_name:
                    in_names.append(name)
            elif alloc.kind == "ExternalOutput":
                shape = tuple(alloc.tensor_shape)
                dtype = mb.dt.np(alloc.dtype)
                out_names.append(name)
                out_avals.append(jax.core.ShapedArray(shape, dtype))
                zero_outs.append(np.zeros(shape, dtype))
        self.in_names = in_names
        self.out_names = out_names
        self.zero_outs = zero_outs
        n_params = len(in_names)
        all_in_names = in_names + out_names
        if partition_name is not None:
            all_in_names.append(partition_name)

        def _body(*args):
            operands = list(args)
            if partition_name is not None:
                operands.append(bass2jax.partition_id_tensor())
            outs = bass2jax._bass_exec_p.bind(
                *operands,
                out_avals=tuple(out_avals),
                in_names=tuple(all_in_names),
                out_names=tuple(out_names),
                lowering_input_output_aliases=(),
                sim_require_finite=True,
                sim_require_nnan=True,
                nc=nc,
            )
            return tuple(outs)

        devices = jax.devices()[:N_CORES]
        self.mesh = Mesh(np.asarray(devices), ("core",))
        in_specs = (PartitionSpec("core"),) * (n_params + len(out_names))
        out_specs = (PartitionSpec("core"),) * len(out_names)
        # no donation: lets us re-run with the same device-resident inputs
        self.fn = jax.jit(
            shard_map(
                _body,
                mesh=self.mesh,
                in_specs=in_specs,
                out_specs=out_specs,
                check_rep=False,
            ),
            keep_unused=True,
        )
        self.out_avals = out_avals

    def stage_inputs(self, in_maps):
        import jax
        from jax.sharding import NamedSharding, PartitionSpec

        sharding = NamedSharding(self.mesh, PartitionSpec("core"))
        args = []
        for name in self.in_names:
            concat = np.concatenate([np.asarray(m[name]) for m in in_maps], axis=0)
            args.append(jax.device_put(concat, sharding))
        for z in self.zero_outs:
            zc = np.zeros((N_CORES * z.shape[0], *z.shape[1:]), z.dtype)
            args.append(jax.device_put(zc, sharding))
        self.args = args

    def run(self):
        outs = self.fn(*self.args)
        self.jax.block_until_ready(outs)
        return outs

    def outputs_to_numpy(self, outs):
        per_core = []
        for c in range(N_CORES):
            per_core.append(
                {
                    name: np.asarray(outs[i]).reshape(
                        N_CORES, *self.out_avals[i].shape
                    )[c]
                    for i, name in enumerate(self.out_names)
                }
            )
        return per_core


_RUNNER_CACHE = {}


def get_runner(tok=TOK_SH):
    if tok not in _RUNNER_CACHE:
        _RUNNER_CACHE[tok] = PjrtRunner(_get_nc(tok))
    return _RUNNER_CACHE[tok]


def _kernel_np_fallback(x, qweight, qzeros, scales, g_idx, bias):
    shifts = (np.arange(8, dtype=np.int64) * 4)[None, :, None]
    wq = ((qweight.astype(np.int64)[:, None, :] >> shifts) & 0xF).reshape(
        IN_F, qweight.shape[1]
    )
    zq = (
        (qzeros.astype(np.int64)[:, :, None] >> shifts.reshape(1, 1, 8)) & 0xF
    ).reshape(qzeros.shape[0], -1) + 1
    w = scales[g_idx] * (wq.astype(np.float32) - zq[g_idx].astype(np.float32))
    return (x.astype(np.float32) @ w + bias).astype(np.float32)


def kernel(x, qweight, qzeros, scales, g_idx, bias):
    x = np.asarray(x)
    qweight = np.asarray(qweight)
    qzeros = np.asarray(qzeros)
    scales = np.asarray(scales)
    g_idx = np.asarray(g_idx)
    bias = np.asarray(bias)

    if not np.array_equal(
        g_idx, (np.arange(IN_F, dtype=np.int64) // GROUPSIZE).astype(g_idx.dtype)
    ):
        return _kernel_np_fallback(x, qweight, qzeros, scales, g_idx, bias)

    runner = get_runner()
    runner.stage_inputs(_shard_inputs(x, qweight, qzeros, scales, bias))
    outs = runner.run()
    return _assemble(runner.outputs_to_numpy(outs))
